# revision 17
# baseline (speedup 1.0000x reference)
"""Trainium2 Bass kernel for nn_AtomicPositionalEncoding.

kernel(**inputs): FULL x [256,1024,4] f32 -> FULL out [256,1024,128] f32.
Shards batch across 8 NeuronCores (32 examples each), one SPMD Bass program.

v2 layout ("i-fold"): partition p owns 8 CONSECUTIVE points of an example
(n = 8p + i).  Column index j = 8b + i (b-major).  This makes the output
DMA fully contiguous per partition (4KB descriptors) and the input load a
strided DMA (no PE shuffle).

Pipeline per group of quads (quad = 4 examples):
  stage1 (global-ish): r/env/radial/prodm via fused scalar_tensor_tensor
  masks = onehot(cls) bf16; prodm2 = prodm^2 bf16
  histT[(q,k),(q,c)] per quad via PE (weights=prodm2 16 cols, moving=masks)
  scaleT = 1/max(sqrt(hist),eps) -> transpose -> squad (blockmask-ed, bf16)
  onehotT via PE transposes; G_B = squad^T @ onehotT; transpose back to G_A
  stats: mean/var over feature axis from PG = prodm*G; PGI = PG*istd
  final per example: X = mask*PGI (S1); out = X + (-mean*istd) (S2); DMA
"""

import os
import sys

import numpy as np

for p in ("/opt/trn_rl_repo", "/root/.axon_site/_ro/trn_rl_repo"):
    if os.path.isdir(p) and p not in sys.path:
        sys.path.insert(0, p)

import concourse.bass as bass
import concourse.bacc as bacc
import concourse.mybir as mybir
from concourse.tile import TileContext

F32 = mybir.dt.float32
BF16 = mybir.dt.bfloat16

EX = 32          # examples per core
NPT = 1024       # points per example
IPC = 8          # points per partition (i index)
NCOL = EX * IPC  # 256 point columns, j = 8*b + i
NQ = 8           # quads of 4 examples
C = 32
K = 4
D = 128
ETA = 4.0
RC = 6.0
Y00 = 0.5 / np.sqrt(np.pi)
C1 = np.sqrt(3.0 / (4.0 * np.pi))
RS = [0.0, 1.5, 3.0, 4.5]

AF = mybir.ActivationFunctionType
OP = mybir.AluOpType

# groups of quads for hist/stats/final batching (pipeline granularity)
GROUPS = [[0], [1], [2, 3], [4, 5, 6, 7]]
# per-example engine pair (S1, S2): 'v'=vector 'g'=gpsimd 'a'=act (S2 only)
EXPLAN = [
    ('v', 'a'), ('g', 'v'), ('v', 'a'), ('g', 'g'),       # quad 0
    ('v', 'a'), ('g', 'v'), ('v', 'a'), ('g', 'v'),       # quad 1
    ('v', 'a'), ('g', 'v'), ('v', 'a'), ('g', 'g'),       # quad 2
    ('v', 'a'), ('g', 'v'), ('v', 'a'), ('g', 'v'),       # quad 3
    ('v', 'a'), ('g', 'v'), ('v', 'a'), ('g', 'g'),       # quad 4
    ('v', 'a'), ('g', 'v'), ('v', 'a'), ('g', 'v'),       # quad 5
    ('v', 'a'), ('g', 'v'), ('g', 'g'), ('g', 'v'),       # quad 6
    ('v', 'a'), ('g', 'v'), ('g', 'g'), ('g', 'v'),       # quad 7
]


def _consts_f32() -> np.ndarray:
    iota32 = np.tile(np.arange(C, dtype=np.float32), (128, 1))          # [128,32]
    rs4 = np.tile(np.array(RS, np.float32), (128, 1))                   # [128,4]
    blockmask = np.zeros((128, 16), dtype=np.float32)                   # [128,16]
    for pp_ in range(128):
        for f in range(16):
            if pp_ // 32 == f // 4:
                blockmask[pp_, f] = 1.0
    ident = np.eye(128, dtype=np.float32)                               # [128,128]
    bconst = np.tile(np.array([np.pi / 2, 0, 0, 0], np.float32), (128, 1))
    return np.concatenate(
        [iota32.ravel(), rs4.ravel(), blockmask.ravel(), ident.ravel(),
         bconst.ravel()]
    )


CF_SIZES = [128 * 32, 128 * 4, 128 * 16, 128 * 128, 128 * 4]
CF_TOTAL = sum(CF_SIZES)


def build_nc() -> bass.Bass:
    nc = bacc.Bacc()
    x_d = nc.dram_tensor("x", [EX, NPT, 4], F32, kind="ExternalInput")
    cf_d = nc.dram_tensor("cf", [CF_TOTAL], F32, kind="ExternalInput")
    out_d = nc.dram_tensor("out", [EX, NPT, D], F32, kind="ExternalOutput")

    with TileContext(nc) as tc:
        with (
            tc.tile_pool(name="persist", bufs=1) as pp,
            tc.tile_pool(name="xpool", bufs=8) as xp,
            tc.tile_pool(name="ohpool", bufs=3) as bb,
            tc.tile_pool(name="outp", bufs=5) as op_,
            tc.tile_pool(name="ph", bufs=1, space="PSUM") as ph,     # histT
            tc.tile_pool(name="poh", bufs=2, space="PSUM") as poh,   # onehotT
            tc.tile_pool(name="pgb", bufs=2, space="PSUM") as pgb,   # G_B
            tc.tile_pool(name="pga", bufs=2, space="PSUM") as pga,   # G_A
            tc.tile_pool(name="psc", bufs=1, space="PSUM") as psc,   # scale
        ):
            ve, act, gp, pe, sy = nc.vector, nc.scalar, nc.gpsimd, nc.tensor, nc.sync

            # ---- constants ----
            offs = np.cumsum([0] + CF_SIZES)
            def cslice(i, shape):
                t = pp.tile(shape, F32, name=f"const{i}", tag=f"const{i}")
                src = cf_d[offs[i]:offs[i + 1]].rearrange("(p f) -> p f", p=shape[0])
                sy.dma_start(t, src)
                return t
            iota32 = cslice(0, [128, 32])
            rs4 = cslice(1, [128, 4])
            blockmask = cslice(2, [128, 16])
            identf = cslice(3, [128, 128])
            bconst = cslice(4, [128, 4])
            ident16 = pp.tile([128, 128], BF16, name="ident16", tag="ident16")
            ve.tensor_copy(ident16, identf)

            # ---- x load: strided DMA into i-fold layout ----
            # x4[p, b, i, c] = x[b, 8p+i, c]
            x_sb = pp.tile([128, NCOL * 4], F32, name="x", tag="x")
            x4 = x_sb.rearrange("p (b i c) -> p b i c", b=EX, i=IPC)
            for h, eng in ((0, sy), (1, act)):
                dst = x4[:, 16 * h:16 * (h + 1)]
                src = x_d[16 * h:16 * (h + 1)].rearrange(
                    "b (p i) c -> p b i c", p=128)
                eng.dma_start(dst, src)
            clsf2 = x_sb.rearrange("p (j c) -> p j c", c=4)[:, :, 3:4] \
                        .rearrange("p j one -> p (j one)")  # [128,256] cls per col

            # ---- persistent per-point tensors ----
            def ptile(name, mult=1, dtype=F32):
                return pp.tile([128, NCOL * mult], dtype, name=name, tag=name)
            r = ptile("r")
            rinv = ptile("rinv")
            env = ptile("env")
            tmp3 = ptile("tmp3", 3)
            u3 = ptile("u3", 3)
            radial = ptile("radial", K)
            m4 = ptile("m4", K)
            prodm = ptile("prodm", K)
            prodm2 = ptile("prodm2", K, BF16)
            masks = ptile("masks", C, BF16)
            g_all = ptile("g_all", K)
            pg = ptile("pg", K)
            pg2 = ptile("pg2", K)
            pgi = ptile("pgi", K)
            mean = ptile("mean")
            msq = ptile("msq")
            var = ptile("var")
            std = ptile("std")
            istd = ptile("istd")
            negmistd = ptile("negmistd")
            squad = pp.tile([128, 128], BF16, name="squad", tag="squad")

            # masks/prodm2 stored (jq, i, q, .) so matmul slices are
            # contiguous; per-point scalars, prodm, g_all, pg stay j=(b,i)
            masks5 = masks.rearrange("p (jq i q c) -> p jq i q c",
                                     jq=NQ, i=IPC, q=4)
            masks3 = masks.rearrange("p (jj c) -> p jj c", c=C)
            prodm25 = prodm2.rearrange("p (jq i q k) -> p jq i q k",
                                       jq=NQ, i=IPC, q=4)
            prodm_jk = prodm.rearrange("p (j k) -> p j k", k=K)
            # permuted view: storage (jq q i k) seen as [p, jq, i, q, k]
            prodm4p = prodm.rearrange("p (jq q i k) -> p jq i q k",
                                      jq=NQ, q=4, i=IPC)
            g_jk = g_all.rearrange("p (j k) -> p j k", k=K)
            g4 = g_all.rearrange("p (jq q i k) -> p jq q i k",
                                 jq=NQ, q=4, i=IPC)
            pg_jk = pg.rearrange("p (j k) -> p j k", k=K)
            pg2_jk = pg2.rearrange("p (j k) -> p j k", k=K)
            pgi_jk = pgi.rearrange("p (j k) -> p j k", k=K)
            # cls permuted to (jq, i, q) column order for the masks STT
            cls_perm = pp.tile([128, NCOL], F32, name="cls_perm", tag="cls_perm")
            ve.tensor_copy(
                cls_perm.rearrange("p (jq i q) -> p jq i q", jq=NQ, i=IPC),
                clsf2.rearrange("p (jq q i) -> p jq i q", jq=NQ, q=4))

            # ---- stage 1: per-point geometry -> prodm ----
            def stage1(c0, c1):
                J = c1 - c0
                xyz = x4.rearrange("p b i c -> p (b i) c")[:, c0:c1, 0:3]
                sq = tmp3.rearrange("p (j c) -> p j c", c=3)[:, c0:c1, :]
                ve.scalar_tensor_tensor(out=sq, in0=xyz, scalar=0.0, in1=xyz,
                                        op0=OP.add, op1=OP.mult)
                ve.tensor_reduce(r[:, c0:c1], sq, mybir.AxisListType.X, OP.add)
                act.activation(r[:, c0:c1], r[:, c0:c1], AF.Sqrt)
                ve.reciprocal(rinv[:, c0:c1], r[:, c0:c1])
                # env = 0.5*cos(pi*min(r,Rc)/Rc) + 0.5
                ve.tensor_scalar(out=env[:, c0:c1], in0=r[:, c0:c1],
                                 scalar1=float(RC), scalar2=None, op0=OP.min)
                act.activation(env[:, c0:c1], env[:, c0:c1], AF.Sin,
                               bias=bconst[:, 0:1], scale=float(-np.pi / RC))
                ve.tensor_scalar(out=env[:, c0:c1], in0=env[:, c0:c1],
                                 scalar1=0.5, scalar2=0.5, op0=OP.mult, op1=OP.add)
                # radial = exp(-eta*(r-Rs)^2), all K at once
                rad = radial.rearrange("p (j k) -> p j k", k=K)[:, c0:c1, :]
                dif = m4.rearrange("p (j k) -> p j k", k=K)[:, c0:c1, :]
                r_b = r[:, c0:c1].unsqueeze(2).broadcast_to([128, J, K])
                rs_b = rs4.unsqueeze(1).broadcast_to([128, J, K])
                ve.scalar_tensor_tensor(out=dif, in0=r_b, scalar=0.0, in1=rs_b,
                                        op0=OP.add, op1=OP.subtract)
                ve.scalar_tensor_tensor(out=dif, in0=dif, scalar=0.0, in1=dif,
                                        op0=OP.add, op1=OP.mult)
                act.activation(rad.rearrange("p j k -> p (j k)"),
                               dif.rearrange("p j k -> p (j k)"),
                               AF.Exp, scale=float(-ETA))
                # m4 = radial * env
                m4v = m4.rearrange("p (j k) -> p j k", k=K)[:, c0:c1, :]
                env_b = env[:, c0:c1].unsqueeze(2).broadcast_to([128, J, K])
                ve.scalar_tensor_tensor(out=m4v, in0=rad, scalar=0.0, in1=env_b,
                                        op0=OP.add, op1=OP.mult)
                # u3 = xyz * rinv
                u3v = u3.rearrange("p (j c) -> p j c", c=3)[:, c0:c1, :]
                rinv_b = rinv[:, c0:c1].unsqueeze(2).broadcast_to([128, J, 3])
                ve.scalar_tensor_tensor(out=u3v, in0=xyz, scalar=0.0, in1=rinv_b,
                                        op0=OP.add, op1=OP.mult)
                # prodm0 = Y00 * m0 ; prodm[1,2,3] = C1 * u[y,z,x] * m[1,2,3]
                pmv = prodm_jk[:, c0:c1, :]
                ve.tensor_scalar(out=pmv[:, :, 0:1], in0=m4v[:, :, 0:1],
                                 scalar1=float(Y00), scalar2=None, op0=OP.mult)
                ve.scalar_tensor_tensor(out=pmv[:, :, 1:3], in0=u3v[:, :, 1:3],
                                        scalar=float(C1), in1=m4v[:, :, 1:3],
                                        op0=OP.mult, op1=OP.mult)
                ve.scalar_tensor_tensor(out=pmv[:, :, 3:4], in0=u3v[:, :, 0:1],
                                        scalar=float(C1), in1=m4v[:, :, 3:4],
                                        op0=OP.mult, op1=OP.mult)

            stage1(0, 32)       # quad 0 express
            stage1(32, NCOL)    # the rest

            # ---- per-group machinery ----
            def do_group(grp):
                L = len(grp)
                jq0 = grp[0]
                c0, c1 = 32 * jq0, 32 * (jq0 + L)
                # masks (bf16 onehot) via 3D STT on permuted cls
                cls_b = cls_perm[:, c0:c1].unsqueeze(2) \
                    .broadcast_to([128, J32 := 32 * L, C])
                iota_b = iota32.unsqueeze(1).broadcast_to([128, J32, C])
                ve.scalar_tensor_tensor(
                    out=masks3[:, c0:c1, :], in0=cls_b, scalar=0.0,
                    in1=iota_b, op0=OP.add, op1=OP.is_equal)
                # prodm2 (bf16 squares) via per-quad 4D permuted tensor_tensor
                for jq in grp:
                    pm4 = prodm4p[:, jq]
                    ve.tensor_tensor(prodm25[:, jq], pm4, pm4, OP.mult)
                # histT per quad: [16=(q,k), 128L], weights=prodm2, moving=masks
                hist_ps = ph.tile([16, 128 * L], F32, name="hist_ps", tag="hist_ps")
                for jx, jq in enumerate(grp):
                    for i in range(IPC):
                        lhsT = prodm25[:, jq, i].rearrange("p q k -> p (q k)")
                        rhs = masks5[:, jq, i].rearrange("p q c -> p (q c)")
                        pe.matmul(hist_ps[:, 128 * jx:128 * (jx + 1)], lhsT, rhs,
                                  start=(i == 0), stop=(i == IPC - 1))
                # scaleT = 1/max(sqrt(hist),1e-12) == min(1/sqrt(hist), 1e12)
                scaleT = xp.tile([16, 128 * L], F32, name="scaleT", tag="scaleT")
                act.activation(scaleT, hist_ps, AF.Sqrt)
                ve.reciprocal(scaleT, scaleT)
                # transpose each quad's scaleT -> [128,16], blockmask, bf16 squad
                scale_ps = psc.tile([128, 16 * L], F32, name="scale_ps", tag="scale_ps")
                for jx in range(L):
                    pe.transpose(scale_ps[:, 16 * jx:16 * (jx + 1)],
                                 scaleT[:, 128 * jx:128 * (jx + 1)],
                                 identf[:16, :16])
                sq_view = squad.rearrange("p (jq f) -> p jq f", f=16)
                bm_b = blockmask.unsqueeze(1).broadcast_to([128, L, 16])
                ve.scalar_tensor_tensor(
                    out=sq_view[:, jq0:jq0 + L],
                    in0=scale_ps.rearrange("p (l f) -> p l f", f=16),
                    scalar=1e12, in1=bm_b, op0=OP.min, op1=OP.mult)

                # per quad: onehotT, G_B, back to G_A
                for jx, jq in enumerate(grp):
                    oh_ps = poh.tile([128, NPT], BF16, name="oh_ps", tag="oh_ps")
                    for i in range(IPC):
                        lhsT = masks5[:, jq, i].rearrange("p q c -> p (q c)")
                        pe.transpose(oh_ps[:, 128 * i:128 * (i + 1)], lhsT, ident16)
                    oh_sb = bb.tile([128, NPT], BF16, name="oh_sb", tag="oh_sb")
                    if jq % 2 == 0:
                        ve.tensor_copy(oh_sb, oh_ps)
                    else:
                        act.copy(oh_sb, oh_ps)
                    gb_sb = xp.tile([16, NPT], F32, name="gb_sb", tag="gb_sb")
                    for h in range(2):
                        gb_ps = pgb.tile([16, 512], F32, name="gb_ps", tag="gb_ps")
                        pe.matmul(gb_ps, sq_view[:, jq, :],
                                  oh_sb[:, 512 * h:512 * (h + 1)],
                                  start=True, stop=True)
                        if h == 0:
                            act.copy(gb_sb[:, 0:512], gb_ps)
                        else:
                            ve.tensor_copy(gb_sb[:, 512:1024], gb_ps)
                    # transpose back: per i -> [128, (q,k)]; then one permuted
                    # 4D copy into j-ordered g_all
                    ga_ps = pga.tile([128, 128], F32, name="ga_ps", tag="ga_ps")
                    for i in range(IPC):
                        pe.transpose(ga_ps[:, 16 * i:16 * (i + 1)],
                                     gb_sb[:, 128 * i:128 * (i + 1)],
                                     identf[:16, :16])
                    ve.tensor_copy(
                        g4[:, jq],
                        ga_ps.rearrange("p (i q k) -> p q i k", i=IPC, q=4))

                # stats over this group's columns
                cs = slice(c0, c1)
                J = c1 - c0
                pgv = pg_jk[:, cs, :]
                ve.scalar_tensor_tensor(out=pgv, in0=prodm_jk[:, cs, :],
                                        scalar=0.0, in1=g_jk[:, cs, :],
                                        op0=OP.add, op1=OP.mult)
                ve.tensor_reduce(mean[:, cs], pgv, mybir.AxisListType.X, OP.add)
                ve.scalar_tensor_tensor(out=pg2_jk[:, cs, :], in0=pgv, scalar=0.0,
                                        in1=pgv, op0=OP.add, op1=OP.mult)
                ve.tensor_reduce(msq[:, cs], pg2_jk[:, cs, :],
                                 mybir.AxisListType.X, OP.add)
                # var*(D-1) = msq - mean^2/D ; istd = 1/(sqrt(var)+1e-6)
                ve.scalar_tensor_tensor(out=var[:, cs], in0=mean[:, cs],
                                        scalar=float(-1.0 / D), in1=mean[:, cs],
                                        op0=OP.mult, op1=OP.mult)
                ve.scalar_tensor_tensor(out=var[:, cs], in0=var[:, cs], scalar=0.0,
                                        in1=msq[:, cs], op0=OP.add, op1=OP.add)
                ve.tensor_scalar(out=var[:, cs], in0=var[:, cs], scalar1=0.0,
                                 scalar2=None, op0=OP.max)
                act.activation(std[:, cs], var[:, cs], AF.Sqrt,
                               scale=float(1.0 / (D - 1)))
                ve.tensor_scalar(out=std[:, cs], in0=std[:, cs], scalar1=1e-6,
                                 scalar2=None, op0=OP.add)
                ve.reciprocal(istd[:, cs], std[:, cs])
                ve.scalar_tensor_tensor(out=negmistd[:, cs], in0=mean[:, cs],
                                        scalar=float(-1.0 / D), in1=istd[:, cs],
                                        op0=OP.mult, op1=OP.mult)
                istd_b = istd[:, cs].unsqueeze(2).broadcast_to([128, J, K])
                ve.scalar_tensor_tensor(out=pgi_jk[:, cs, :], in0=pgv, scalar=0.0,
                                        in1=istd_b, op0=OP.add, op1=OP.mult)

                # final per example
                for jx, jq in enumerate(grp):
                    for q in range(4):
                        b = 4 * jq + q
                        s1e, s2e = EXPLAN[b]
                        out_ex = op_.tile([128, IPC * D], F32,
                                          name="out_ex", tag="out_ex")
                        xt = xp.tile([128, IPC * D], F32, name="xt", tag="xt")
                        x4v = xt.rearrange("p (i k c) -> p i k c", i=IPC, k=K)
                        mask_b = masks5[:, jq, :, q, :].unsqueeze(2) \
                            .broadcast_to([128, IPC, K, C])
                        pgi_b = pgi_jk[:, 8 * b:8 * b + 8, :].unsqueeze(3) \
                            .broadcast_to([128, IPC, K, C])
                        e1 = ve if s1e == 'v' else gp
                        e1.tensor_tensor(x4v, mask_b, pgi_b, OP.mult)
                        if s2e == 'a':
                            for i in range(IPC):
                                j = 8 * b + i
                                act.activation(out_ex[:, D * i:D * (i + 1)],
                                               xt[:, D * i:D * (i + 1)],
                                               AF.Identity,
                                               bias=negmistd[:, j:j + 1], scale=1.0)
                        else:
                            nm_b = negmistd[:, 8 * b:8 * b + 8].unsqueeze(2) \
                                .broadcast_to([128, IPC, D])
                            o3 = out_ex.rearrange("p (i d) -> p i d", d=D)
                            x3 = xt.rearrange("p (i d) -> p i d", d=D)
                            if s2e == 'v':
                                ve.scalar_tensor_tensor(
                                    out=o3, in0=x3, scalar=0.0, in1=nm_b,
                                    op0=OP.add, op1=OP.add)
                            else:
                                gp.tensor_tensor(o3, x3, nm_b, OP.add)
                        dst = out_d[b].rearrange("(p i) d -> p (i d)", p=128)
                        sy.dma_start(dst, out_ex)

            for grp in GROUPS:
                do_group(grp)

    if not nc.is_finalized():
        nc.finalize()
    return nc


_NC = None


def _get_nc():
    global _NC
    if _NC is None:
        _NC = build_nc()
    return _NC


def kernel(x: np.ndarray) -> np.ndarray:
    from concourse.bass_utils import run_bass_kernel_spmd

    x = np.ascontiguousarray(np.asarray(x, dtype=np.float32))
    B = x.shape[0]
    n_cores = 8
    per = B // n_cores
    cf = _consts_f32()
    nc = _get_nc()
    in_maps = [
        {"x": x[i * per:(i + 1) * per], "cf": cf} for i in range(n_cores)
    ]
    res = run_bass_kernel_spmd(nc, in_maps, core_ids=list(range(n_cores)))
    return np.concatenate([r["out"] for r in res.results], axis=0)


if __name__ == "__main__":
    from concourse.bass_interp import CoreSim

    rng = np.random.default_rng(0)
    x = (rng.standard_normal((EX, NPT, 4)) * 2.0).astype(np.float32)
    x[..., 3] = rng.integers(0, C, size=(EX, NPT)).astype(np.float32)
    nc = build_nc()
    sim = CoreSim(nc)
    sim.tensor("x")[:] = x
    sim.tensor("cf")[:] = _consts_f32()
    sim.simulate()
    got = np.array(sim.tensor("out"))

    xyz = x[..., :3]; clsf_ = x[..., 3]
    r = np.sqrt((xyz * xyz).sum(-1)); rinv = 1.0 / r
    radial = np.exp(-ETA * (np.array(RS, np.float32)[None, None] - r[..., None]) ** 2)
    env = 0.5 * np.cos(np.pi * np.minimum(r, RC) / RC) + 0.5
    sh = np.stack([np.full_like(r, Y00), C1 * xyz[..., 1] * rinv,
                   C1 * xyz[..., 2] * rinv, C1 * xyz[..., 0] * rinv], -1)
    prod = sh * radial * env[..., None]
    onehot = (clsf_[..., None] == np.arange(C, dtype=np.float32)).astype(np.float32)
    pos = (prod[..., :, None] * onehot[..., None, :]).reshape(EX, NPT, D)
    norm = np.sqrt((pos * pos).sum(1, keepdims=True))
    pos = pos / np.maximum(norm, 1e-12)
    mean_ = pos.mean(-1, keepdims=True)
    std_ = pos.std(-1, ddof=1, keepdims=True)
    want = (pos - mean_) / (std_ + 1e-6)
    print("sim absmax err:", np.abs(got - want).max(), "ref absmax:", np.abs(want).max())


# revision 30
# speedup vs baseline: 1.0626x; 1.0626x over previous
"""Trainium2 Bass kernel for nn_AtomicPositionalEncoding.

kernel(**inputs): FULL x [256,1024,4] f32 -> FULL out [256,1024,128] f32.
Shards batch across 8 NeuronCores (32 examples each), one SPMD Bass program.

v2 layout ("i-fold"): partition p owns 8 CONSECUTIVE points of an example
(n = 8p + i).  Column index j = 8b + i (b-major).  This makes the output
DMA fully contiguous per partition (4KB descriptors) and the input load a
strided DMA (no PE shuffle).

Pipeline per group of quads (quad = 4 examples):
  stage1 (global-ish): r/env/radial/prodm via fused scalar_tensor_tensor
  masks = onehot(cls) bf16; prodm2 = prodm^2 bf16
  histT[(q,k),(q,c)] per quad via PE (weights=prodm2 16 cols, moving=masks)
  scaleT = 1/max(sqrt(hist),eps) -> transpose -> squad (blockmask-ed, bf16)
  onehotT via PE transposes; G_B = squad^T @ onehotT; transpose back to G_A
  stats: mean/var over feature axis from PG = prodm*G; PGI = PG*istd
  final per example: X = mask*PGI (S1); out = X + (-mean*istd) (S2); DMA
"""

import os
import sys

import numpy as np

for p in ("/opt/trn_rl_repo", "/root/.axon_site/_ro/trn_rl_repo"):
    if os.path.isdir(p) and p not in sys.path:
        sys.path.insert(0, p)

import concourse.bass as bass
import concourse.bacc as bacc
import concourse.mybir as mybir
from concourse.tile import TileContext

F32 = mybir.dt.float32
BF16 = mybir.dt.bfloat16

EX = 32          # examples per core
NPT = 1024       # points per example
IPC = 8          # points per partition (i index)
NCOL = EX * IPC  # 256 point columns, j = 8*b + i
NQ = 8           # quads of 4 examples
C = 32
K = 4
D = 128
ETA = 4.0
RC = 6.0
Y00 = 0.5 / np.sqrt(np.pi)
C1 = np.sqrt(3.0 / (4.0 * np.pi))
RS = [0.0, 1.5, 3.0, 4.5]

AF = mybir.ActivationFunctionType
OP = mybir.AluOpType

# groups of quads for hist/stats/final batching (pipeline granularity)
GROUPS = [[0], [1], [2, 3], [4, 5, 6, 7]]
# per-example engine pair (S1, S2): 'v'=vector 'g'=gpsimd 'a'=act (S2 only)
# budget: S1 ve x19 gp x13; S2 act x18 ve x2 gp x12
EXPLAN = [
    ('v', 'a'), ('v', 'v'), ('g', 'a'), ('g', 'g'),       # quad 0
    ('v', 'a'), ('v', 'a'), ('g', 'g'), ('v', 'a'),       # quad 1
    ('v', 'a'), ('g', 'g'), ('v', 'a'), ('g', 'g'),       # quad 2
    ('v', 'a'), ('g', 'g'), ('v', 'a'), ('g', 'g'),       # quad 3
    ('v', 'a'), ('g', 'g'), ('v', 'a'), ('g', 'g'),       # quad 4
    ('v', 'a'), ('g', 'g'), ('v', 'a'), ('g', 'g'),       # quad 5
    ('v', 'a'), ('g', 'g'), ('v', 'a'), ('v', 'a'),       # quad 6
    ('v', 'a'), ('g', 'g'), ('v', 'v'), ('v', 'a'),       # quad 7
]


def _consts_f32() -> np.ndarray:
    iota32 = np.tile(np.arange(C, dtype=np.float32), (128, 1))          # [128,32]
    rs4 = np.tile(np.array(RS, np.float32), (128, 1))                   # [128,4]
    blockmask = np.zeros((128, 16), dtype=np.float32)                   # [128,16]
    for pp_ in range(128):
        for f in range(16):
            if pp_ // 32 == f // 4:
                blockmask[pp_, f] = 1.0
    ident = np.eye(128, dtype=np.float32)                               # [128,128]
    bconst = np.tile(np.array([np.pi / 2, 0, 0, 0], np.float32), (128, 1))
    return np.concatenate(
        [iota32.ravel(), rs4.ravel(), blockmask.ravel(), ident.ravel(),
         bconst.ravel()]
    )


CF_SIZES = [128 * 32, 128 * 4, 128 * 16, 128 * 128, 128 * 4]
CF_TOTAL = sum(CF_SIZES)


def build_nc() -> bass.Bass:
    nc = bacc.Bacc()
    x_d = nc.dram_tensor("x", [EX, NPT, 4], F32, kind="ExternalInput")
    cf_d = nc.dram_tensor("cf", [CF_TOTAL], F32, kind="ExternalInput")
    out_d = nc.dram_tensor("out", [EX, NPT, D], F32, kind="ExternalOutput")

    with TileContext(nc) as tc:
        with (
            tc.tile_pool(name="persist", bufs=1) as pp,
            tc.tile_pool(name="xpool", bufs=8) as xp,
            tc.tile_pool(name="ohpool", bufs=3) as bb,
            tc.tile_pool(name="outp", bufs=5) as op_,
            tc.tile_pool(name="ph", bufs=1, space="PSUM") as ph,     # histT
            tc.tile_pool(name="poh", bufs=2, space="PSUM") as poh,   # onehotT
            tc.tile_pool(name="pgb", bufs=2, space="PSUM") as pgb,   # G_B
            tc.tile_pool(name="pga", bufs=2, space="PSUM") as pga,   # G_A
            tc.tile_pool(name="psc", bufs=1, space="PSUM") as psc,   # scale
        ):
            ve, act, gp, pe, sy = nc.vector, nc.scalar, nc.gpsimd, nc.tensor, nc.sync

            # ---- constants ----
            offs = np.cumsum([0] + CF_SIZES)
            def cslice(i, shape):
                t = pp.tile(shape, F32, name=f"const{i}", tag=f"const{i}")
                src = cf_d[offs[i]:offs[i + 1]].rearrange("(p f) -> p f", p=shape[0])
                sy.dma_start(t, src)
                return t
            iota32 = cslice(0, [128, 32])
            rs4 = cslice(1, [128, 4])
            blockmask = cslice(2, [128, 16])
            identf = cslice(3, [128, 128])
            bconst = cslice(4, [128, 4])
            ident16 = pp.tile([128, 128], BF16, name="ident16", tag="ident16")
            ve.tensor_copy(ident16, identf)

            # ---- x load: strided DMA into i-fold layout ----
            # x4[p, b, i, c] = x[b, 8p+i, c]
            x_sb = pp.tile([128, NCOL * 4], F32, name="x", tag="x")
            x4 = x_sb.rearrange("p (b i c) -> p b i c", b=EX, i=IPC)
            for h, eng in ((0, sy), (1, act)):
                dst = x4[:, 16 * h:16 * (h + 1)]
                src = x_d[16 * h:16 * (h + 1)].rearrange(
                    "b (p i) c -> p b i c", p=128)
                eng.dma_start(dst, src)
            clsf2 = x_sb.rearrange("p (j c) -> p j c", c=4)[:, :, 3:4] \
                        .rearrange("p j one -> p (j one)")  # [128,256] cls per col

            # ---- persistent per-point tensors ----
            def ptile(name, mult=1, dtype=F32):
                return pp.tile([128, NCOL * mult], dtype, name=name, tag=name)
            r = ptile("r")
            rinv = ptile("rinv")
            env = ptile("env")
            tmp3 = ptile("tmp3", 3)
            u3 = ptile("u3", 3)
            radial = ptile("radial", K)
            m4 = ptile("m4", K)
            prodm = ptile("prodm", K)
            prodm2 = ptile("prodm2", K, BF16)
            masks = ptile("masks", C, BF16)
            g_all = ptile("g_all", K)
            pg = ptile("pg", K)
            pg2 = ptile("pg2", K)
            pgi = ptile("pgi", K)
            mean = ptile("mean")
            msq = ptile("msq")
            var = ptile("var")
            std = ptile("std")
            istd = ptile("istd")
            negmistd = ptile("negmistd")
            # 32 zero pad columns so every quad has a 32-wide weight window;
            # zero it all once so windows may read not-yet-written quads
            squad = pp.tile([128, 160], BF16, name="squad", tag="squad")
            ve.memset(squad, 0.0)

            # masks/prodm2 stored (jq, i, q, .) so matmul slices are
            # contiguous; per-point scalars, prodm, g_all, pg stay j=(b,i)
            masks5 = masks.rearrange("p (jq i q c) -> p jq i q c",
                                     jq=NQ, i=IPC, q=4)
            masks3 = masks.rearrange("p (jj c) -> p jj c", c=C)
            prodm25 = prodm2.rearrange("p (jq i q k) -> p jq i q k",
                                       jq=NQ, i=IPC, q=4)
            prodm_jk = prodm.rearrange("p (j k) -> p j k", k=K)
            prodm2j = ptile("prodm2j", K, BF16)
            prodm2j_jk = prodm2j.rearrange("p (j k) -> p j k", k=K)
            # permuted view: storage (jq q i k) seen as [p, jq, i, q, k]
            prodm2j4p = prodm2j.rearrange("p (jq q i k) -> p jq i q k",
                                          jq=NQ, q=4, i=IPC)
            g_jk = g_all.rearrange("p (j k) -> p j k", k=K)
            g4 = g_all.rearrange("p (jq q i k) -> p jq q i k",
                                 jq=NQ, q=4, i=IPC)
            pg_jk = pg.rearrange("p (j k) -> p j k", k=K)
            pg2_jk = pg2.rearrange("p (j k) -> p j k", k=K)
            pgi_jk = pgi.rearrange("p (j k) -> p j k", k=K)
            # cls permuted to (jq, i, q) column order for the masks STT
            cls_perm = pp.tile([128, NCOL], F32, name="cls_perm", tag="cls_perm")
            ve.tensor_copy(
                cls_perm.rearrange("p (jq i q) -> p jq i q", jq=NQ, i=IPC),
                clsf2.rearrange("p (jq q i) -> p jq i q", jq=NQ, q=4))

            # ---- stage 1: per-point geometry -> prodm ----
            def stage1(c0, c1):
                J = c1 - c0
                xyz = x4.rearrange("p b i c -> p (b i) c")[:, c0:c1, 0:3]
                sq = tmp3.rearrange("p (j c) -> p j c", c=3)[:, c0:c1, :]
                ve.scalar_tensor_tensor(out=sq, in0=xyz, scalar=0.0, in1=xyz,
                                        op0=OP.add, op1=OP.mult)
                ve.tensor_reduce(r[:, c0:c1], sq, mybir.AxisListType.X, OP.add)
                act.activation(r[:, c0:c1], r[:, c0:c1], AF.Sqrt)
                ve.reciprocal(rinv[:, c0:c1], r[:, c0:c1])
                # env = 0.5*cos(pi*min(r,Rc)/Rc) + 0.5
                ve.tensor_scalar(out=env[:, c0:c1], in0=r[:, c0:c1],
                                 scalar1=float(RC), scalar2=None, op0=OP.min)
                act.activation(env[:, c0:c1], env[:, c0:c1], AF.Sin,
                               bias=bconst[:, 0:1], scale=float(-np.pi / RC))
                ve.tensor_scalar(out=env[:, c0:c1], in0=env[:, c0:c1],
                                 scalar1=0.5, scalar2=0.5, op0=OP.mult, op1=OP.add)
                # radial = exp(-eta*(r-Rs)^2), all K at once
                rad = radial.rearrange("p (j k) -> p j k", k=K)[:, c0:c1, :]
                dif = m4.rearrange("p (j k) -> p j k", k=K)[:, c0:c1, :]
                r_b = r[:, c0:c1].unsqueeze(2).broadcast_to([128, J, K])
                rs_b = rs4.unsqueeze(1).broadcast_to([128, J, K])
                ve.scalar_tensor_tensor(out=dif, in0=r_b, scalar=0.0, in1=rs_b,
                                        op0=OP.add, op1=OP.subtract)
                ve.scalar_tensor_tensor(out=dif, in0=dif, scalar=0.0, in1=dif,
                                        op0=OP.add, op1=OP.mult)
                act.activation(rad.rearrange("p j k -> p (j k)"),
                               dif.rearrange("p j k -> p (j k)"),
                               AF.Exp, scale=float(-ETA))
                # m4 = radial * env
                m4v = m4.rearrange("p (j k) -> p j k", k=K)[:, c0:c1, :]
                env_b = env[:, c0:c1].unsqueeze(2).broadcast_to([128, J, K])
                ve.scalar_tensor_tensor(out=m4v, in0=rad, scalar=0.0, in1=env_b,
                                        op0=OP.add, op1=OP.mult)
                # u3 = xyz * rinv
                u3v = u3.rearrange("p (j c) -> p j c", c=3)[:, c0:c1, :]
                rinv_b = rinv[:, c0:c1].unsqueeze(2).broadcast_to([128, J, 3])
                ve.scalar_tensor_tensor(out=u3v, in0=xyz, scalar=0.0, in1=rinv_b,
                                        op0=OP.add, op1=OP.mult)
                # prodm0 = Y00 * m0 ; prodm[1,2,3] = C1 * u[y,z,x] * m[1,2,3]
                pmv = prodm_jk[:, c0:c1, :]
                ve.tensor_scalar(out=pmv[:, :, 0:1], in0=m4v[:, :, 0:1],
                                 scalar1=float(Y00), scalar2=None, op0=OP.mult)
                ve.scalar_tensor_tensor(out=pmv[:, :, 1:3], in0=u3v[:, :, 1:3],
                                        scalar=float(C1), in1=m4v[:, :, 1:3],
                                        op0=OP.mult, op1=OP.mult)
                ve.scalar_tensor_tensor(out=pmv[:, :, 3:4], in0=u3v[:, :, 0:1],
                                        scalar=float(C1), in1=m4v[:, :, 3:4],
                                        op0=OP.mult, op1=OP.mult)

            stage1(0, 64)       # quads 0-1 express
            stage1(64, NCOL)    # the rest

            # ---- per-group machinery ----
            def do_group(grp):
                L = len(grp)
                jq0 = grp[0]
                c0, c1 = 32 * jq0, 32 * (jq0 + L)
                # masks (bf16 onehot) via 3D STT on permuted cls
                cls_b = cls_perm[:, c0:c1].unsqueeze(2) \
                    .broadcast_to([128, J32 := 32 * L, C])
                iota_b = iota32.unsqueeze(1).broadcast_to([128, J32, C])
                ve.scalar_tensor_tensor(
                    out=masks3[:, c0:c1, :], in0=cls_b, scalar=0.0,
                    in1=iota_b, op0=OP.add, op1=OP.is_equal)
                # prodm2: square in j-order (cheap STT), then per-quad
                # permuted bf16 copies into i-major layout for the PE
                pmj = prodm_jk[:, c0:c1, :]
                ve.scalar_tensor_tensor(out=prodm2j_jk[:, c0:c1, :], in0=pmj,
                                        scalar=0.0, in1=pmj,
                                        op0=OP.add, op1=OP.mult)
                for jq in grp:
                    ve.tensor_copy(prodm25[:, jq], prodm2j4p[:, jq])
                # histT per quad: [16=(q,k), 128L], weights=prodm2, moving=masks
                hist_ps = ph.tile([16, 128 * L], F32, name="hist_ps", tag="hist_ps")
                for jx, jq in enumerate(grp):
                    for i in range(IPC):
                        lhsT = prodm25[:, jq, i].rearrange("p q k -> p (q k)")
                        rhs = masks5[:, jq, i].rearrange("p q c -> p (q c)")
                        pe.matmul(hist_ps[:, 128 * jx:128 * (jx + 1)], lhsT, rhs,
                                  start=(i == 0), stop=(i == IPC - 1))
                # sqrt(hist) -> transpose -> 1/x on the narrow side
                # squad = min(1/sqrt(hist), 1e12) * blockmask
                scaleT = xp.tile([16, 128 * L], F32, name="scaleT", tag="scaleT")
                act.activation(scaleT, hist_ps, AF.Sqrt)
                scale_ps = psc.tile([128, 16 * L], F32, name="scale_ps", tag="scale_ps")
                for jx in range(L):
                    pe.transpose(scale_ps[:, 16 * jx:16 * (jx + 1)],
                                 scaleT[:, 128 * jx:128 * (jx + 1)],
                                 identf[:16, :16])
                scinv = xp.tile([128, 16 * L], F32, name="scinv", tag="scinv")
                ve.reciprocal(scinv, scale_ps)
                sq_view = squad[:, 0:128].rearrange("p (jq f) -> p jq f", f=16)
                bm_b = blockmask.unsqueeze(1).broadcast_to([128, L, 16])
                ve.scalar_tensor_tensor(
                    out=sq_view[:, jq0:jq0 + L],
                    in0=scinv.rearrange("p (l f) -> p l f", f=16),
                    scalar=1e12, in1=bm_b, op0=OP.min, op1=OP.mult)

                # per pair of quads: onehotT, G_B (partition-packed), G_A
                pairs = [grp[i:i + 2] for i in range(0, L, 2)]
                for pair in pairs:
                    ohs = []
                    for jq in pair:
                        oh_ps = poh.tile([128, NPT], BF16, name="oh_ps",
                                         tag="oh_ps")
                        for i in range(IPC):
                            lhsT = masks5[:, jq, i].rearrange("p q c -> p (q c)")
                            pe.transpose(oh_ps[:, 128 * i:128 * (i + 1)],
                                         lhsT, ident16)
                        oh_sb = bb.tile([128, NPT], BF16, name="oh_sb",
                                        tag="oh_sb")
                        if jq % 2 == 0:
                            ve.tensor_copy(oh_sb, oh_ps)
                        else:
                            act.copy(oh_sb, oh_ps)
                        ohs.append(oh_sb)
                    # 32-col squad windows keep the packed PSUM fully written
                    P = 32 * len(pair)
                    gb_sb = xp.tile([P, NPT], F32, name="gb_sb", tag="gb_sb")
                    for h in range(2):
                        gb_ps = pgb.tile([P, 512], F32, name="gb_ps", tag="gb_ps")
                        for px, jq in enumerate(pair):
                            pe.matmul(gb_ps[32 * px:32 * px + 32, :],
                                      squad[:, 16 * jq:16 * jq + 32],
                                      ohs[px][:, 512 * h:512 * (h + 1)],
                                      start=True, stop=True)
                        act.copy(gb_sb[:, 512 * h:512 * (h + 1)], gb_ps)
                    for px, jq in enumerate(pair):
                        ga_ps = pga.tile([128, 128], F32, name="ga_ps",
                                         tag="ga_ps")
                        p0 = 32 * px
                        for i in range(IPC):
                            pe.transpose(
                                ga_ps[:, 16 * i:16 * (i + 1)],
                                gb_sb[p0:p0 + 16, 128 * i:128 * (i + 1)],
                                identf[p0:p0 + 16, p0:p0 + 16])
                        ve.tensor_copy(
                            g4[:, jq],
                            ga_ps.rearrange("p (i q k) -> p q i k", i=IPC, q=4))

                # stats over this group's columns
                cs = slice(c0, c1)
                J = c1 - c0
                pgv = pg_jk[:, cs, :]
                ve.scalar_tensor_tensor(out=pgv, in0=prodm_jk[:, cs, :],
                                        scalar=0.0, in1=g_jk[:, cs, :],
                                        op0=OP.add, op1=OP.mult)
                ve.tensor_reduce(mean[:, cs], pgv, mybir.AxisListType.X, OP.add)
                ve.scalar_tensor_tensor(out=pg2_jk[:, cs, :], in0=pgv, scalar=0.0,
                                        in1=pgv, op0=OP.add, op1=OP.mult)
                ve.tensor_reduce(msq[:, cs], pg2_jk[:, cs, :],
                                 mybir.AxisListType.X, OP.add)
                # var*(D-1) = msq - mean^2/D ; istd = 1/(sqrt(var)+1e-6)
                ve.scalar_tensor_tensor(out=var[:, cs], in0=mean[:, cs],
                                        scalar=float(-1.0 / D), in1=mean[:, cs],
                                        op0=OP.mult, op1=OP.mult)
                ve.scalar_tensor_tensor(out=var[:, cs], in0=var[:, cs], scalar=0.0,
                                        in1=msq[:, cs], op0=OP.add, op1=OP.add)
                ve.tensor_scalar(out=var[:, cs], in0=var[:, cs], scalar1=0.0,
                                 scalar2=None, op0=OP.max)
                act.activation(std[:, cs], var[:, cs], AF.Sqrt,
                               scale=float(1.0 / (D - 1)))
                ve.tensor_scalar(out=std[:, cs], in0=std[:, cs], scalar1=1e-6,
                                 scalar2=None, op0=OP.add)
                ve.reciprocal(istd[:, cs], std[:, cs])
                ve.scalar_tensor_tensor(out=negmistd[:, cs], in0=mean[:, cs],
                                        scalar=float(-1.0 / D), in1=istd[:, cs],
                                        op0=OP.mult, op1=OP.mult)
                istd_b = istd[:, cs].unsqueeze(2).broadcast_to([128, J, K])
                ve.scalar_tensor_tensor(out=pgi_jk[:, cs, :], in0=pgv, scalar=0.0,
                                        in1=istd_b, op0=OP.add, op1=OP.mult)

                # final per example
                for jx, jq in enumerate(grp):
                    for q in range(4):
                        b = 4 * jq + q
                        s1e, s2e = EXPLAN[b]
                        out_ex = op_.tile([128, IPC * D], F32,
                                          name="out_ex", tag="out_ex")
                        xt = xp.tile([128, IPC * D], F32, name="xt", tag="xt")
                        x4v = xt.rearrange("p (i k c) -> p i k c", i=IPC, k=K)
                        mask_b = masks5[:, jq, :, q, :].unsqueeze(2) \
                            .broadcast_to([128, IPC, K, C])
                        pgi_b = pgi_jk[:, 8 * b:8 * b + 8, :].unsqueeze(3) \
                            .broadcast_to([128, IPC, K, C])
                        e1 = ve if s1e == 'v' else gp
                        e1.tensor_tensor(x4v, mask_b, pgi_b, OP.mult)
                        if s2e == 'a':
                            for i in range(IPC):
                                j = 8 * b + i
                                act.activation(out_ex[:, D * i:D * (i + 1)],
                                               xt[:, D * i:D * (i + 1)],
                                               AF.Identity,
                                               bias=negmistd[:, j:j + 1], scale=1.0)
                        else:
                            nm_b = negmistd[:, 8 * b:8 * b + 8].unsqueeze(2) \
                                .broadcast_to([128, IPC, D])
                            o3 = out_ex.rearrange("p (i d) -> p i d", d=D)
                            x3 = xt.rearrange("p (i d) -> p i d", d=D)
                            if s2e == 'v':
                                ve.scalar_tensor_tensor(
                                    out=o3, in0=x3, scalar=0.0, in1=nm_b,
                                    op0=OP.add, op1=OP.add)
                            else:
                                gp.tensor_tensor(o3, x3, nm_b, OP.add)
                        dst = out_d[b].rearrange("(p i) d -> p (i d)", p=128)
                        sy.dma_start(dst, out_ex)

            for grp in GROUPS:
                do_group(grp)

    if not nc.is_finalized():
        nc.finalize()
    return nc


_NC = None


def _get_nc():
    global _NC
    if _NC is None:
        _NC = build_nc()
    return _NC


def kernel(x: np.ndarray) -> np.ndarray:
    from concourse.bass_utils import run_bass_kernel_spmd

    x = np.ascontiguousarray(np.asarray(x, dtype=np.float32))
    B = x.shape[0]
    n_cores = 8
    per = B // n_cores
    cf = _consts_f32()
    nc = _get_nc()
    in_maps = [
        {"x": x[i * per:(i + 1) * per], "cf": cf} for i in range(n_cores)
    ]
    res = run_bass_kernel_spmd(nc, in_maps, core_ids=list(range(n_cores)))
    return np.concatenate([r["out"] for r in res.results], axis=0)


if __name__ == "__main__":
    from concourse.bass_interp import CoreSim

    rng = np.random.default_rng(0)
    x = (rng.standard_normal((EX, NPT, 4)) * 2.0).astype(np.float32)
    x[..., 3] = rng.integers(0, C, size=(EX, NPT)).astype(np.float32)
    nc = build_nc()
    sim = CoreSim(nc)
    sim.tensor("x")[:] = x
    sim.tensor("cf")[:] = _consts_f32()
    sim.simulate()
    got = np.array(sim.tensor("out"))

    xyz = x[..., :3]; clsf_ = x[..., 3]
    r = np.sqrt((xyz * xyz).sum(-1)); rinv = 1.0 / r
    radial = np.exp(-ETA * (np.array(RS, np.float32)[None, None] - r[..., None]) ** 2)
    env = 0.5 * np.cos(np.pi * np.minimum(r, RC) / RC) + 0.5
    sh = np.stack([np.full_like(r, Y00), C1 * xyz[..., 1] * rinv,
                   C1 * xyz[..., 2] * rinv, C1 * xyz[..., 0] * rinv], -1)
    prod = sh * radial * env[..., None]
    onehot = (clsf_[..., None] == np.arange(C, dtype=np.float32)).astype(np.float32)
    pos = (prod[..., :, None] * onehot[..., None, :]).reshape(EX, NPT, D)
    norm = np.sqrt((pos * pos).sum(1, keepdims=True))
    pos = pos / np.maximum(norm, 1e-12)
    mean_ = pos.mean(-1, keepdims=True)
    std_ = pos.std(-1, ddof=1, keepdims=True)
    want = (pos - mean_) / (std_ + 1e-6)
    print("sim absmax err:", np.abs(got - want).max(), "ref absmax:", np.abs(want).max())
